# revision 49
# baseline (speedup 1.0000x reference)
"""DeltaNet forward kernel for 8 Trainium2 NeuronCores — v2 (restructured).

Problem (hardcoded): hidden_states [B=4, T=2048, D=1024], H=4 heads, Dh=256,
causal depthwise conv K=4 + silu on q/k/v projections, q/k l2-normalized per
head (q scaled Dh^-0.5), delta-rule recurrence over T, per-head RMSNorm,
merge heads, out = o @ Wo.

Sharding: core c -> batch c//2, head group c%2 (projection columns
[512*(c%2), 512*(c%2)+512)). Host sums the two row-parallel partials.

v2 design vs baseline:
- Projections and output matmul in fp8e4m3 + DoubleRow (0.5 cyc/row).
  Weights pre-scaled x64 host-side; 1/64 folded into conv weights / out copy.
- Depthwise conv as diagonal-matrix fp16 matmuls on PE for q/k (psum
  accumulate over shifted rhs views); v's conv on DVE (TSM 4x + TT-add tree).
- Phase B (delta recurrence, chunk C=128) in U-form: per-chunk S-independent
  precompute (parallel across all 32 head-chunks):
    KK = K K^T; 64-block masks D, B=D^T, FnT=-L^T;
    R^T = (I+B^8)(I+B^4)(I-B+B^2-B^3) masked doubling;
    M = (I+A)^{-1} via 2-block substitution on identity RHS;
    Mtn = -M^T; GTn_j = -K_j^T M^T; u0n = -M V; Pat = triu_incl(K Q^T).
  serial chain per head: U' = U0 - G S (3 mm); S += K^T U' (2 mm, persistent
  psum); O = Q S_old + Pat^T U' (3 mm) -> RMS -> transpose -> oT8 (fp8).
- Output projection fp8-DR from oT8, fp16 DMA out, f32 partial sum on host.
"""

import numpy as np

B, T, D = 4, 2048, 1024
H = 4
DH = D // H          # 256
CONV_K = 4
EPS = 1e-5
NCORES = 8
CG = 512             # columns per core (2 heads)
C = 128              # recurrence chunk length
NCHUNK = T // C      # 16
PAD = 4              # front zero padding on time axis for the causal conv
TOKB = 512           # token block
NB = T // TOKB       # 4
KP = D // 256        # 4 DR contraction pairs for D=1024
KT8 = D // 128       # 8 fp16 contraction tiles
CT = CG // 128       # 4 column tiles per core
WSCALE = 1.0         # no fp8 pre-scale in the fp16 path

_CACHE = {}
SILU_NATIVE = True  # CoreSim lacks Silu; set False for simulation runs
DEBUG_DUMP = False
PHASE_LIMIT = 0  # 1: only phase A, 2: A+B (no C); 0: full
STAGE_LIMIT = 99


def _build_bass():
    import concourse.bass as bass  # noqa: F401
    import concourse.bacc as bacc
    import concourse.mybir as mybir
    import concourse.tile as tile

    dt = mybir.dt
    nc = bacc.Bacc("TRN2", target_bir_lowering=False, debug=False)

    x8 = nc.dram_tensor("x8", [KT8, 128, T], dt.float16, kind="ExternalInput")
    w8 = nc.dram_tensor("w8", [3 * KT8, 128, CG], dt.float16,
                        kind="ExternalInput")
    wo8 = nc.dram_tensor("wo8", [4, 128, D], dt.float16,
                         kind="ExternalInput")
    diag = nc.dram_tensor("diag", [2 * CT, 128, CONV_K * 128], dt.float16,
                          kind="ExternalInput")
    cwv = nc.dram_tensor("cwv", [CT, 128, CONV_K], dt.float32,
                         kind="ExternalInput")
    consts = nc.dram_tensor("consts", [128, 7 * 128], dt.float16,
                            kind="ExternalInput")
    out = nc.dram_tensor("out", [T, D], dt.float16, kind="ExternalOutput")
    dbg = None
    if DEBUG_DUMP:
        dbg = nc.dram_tensor("dbg", [14, 128, T], dt.float32,
                             kind="ExternalOutput")

    with tile.TileContext(nc) as tc:
        _body(nc, tc, mybir, x8, w8, wo8, diag, cwv, consts, out, dbg)

    nc.compile()
    return nc


def _body(nc, tc, mybir, x8, w8, wo8, diag, cwv, consts, out, dbg=None):
    dt = mybir.dt
    AF = mybir.ActivationFunctionType
    PM = mybir.MatmulPerfMode.DoubleRow
    fp32 = dt.float32
    f16 = dt.float16
    f8 = dt.float8e4

    x8_t = x8.ap()          # [KP, 128, 2T]
    w8_t = w8.ap()          # [3KP, 128, 2CG]
    wo8_t = wo8.ap()        # [2, 128, 2D]
    diag_t = diag.ap()      # [2CT, 128, 4*128]
    cwv_t = cwv.ap()        # [CT, 128, 4]
    out_t = out.ap().rearrange("(n p) c -> n p c", p=128)   # [16,128,D]

    with tc.tile_pool(name="persist", bufs=1) as persist, \
         tc.tile_pool(name="qkvp", bufs=1) as qkvp, \
         tc.tile_pool(name="otp", bufs=1) as otp:

        # ---------------- constants / weights ----------------
        cons = persist.tile([128, 7 * 128], f16, name="cons", tag="cons")
        ident = cons[:, 0:128]
        identN = cons[:, 128:256]       # -I
        bdl64 = cons[:, 256:384]        # strict lower within 64-blocks, +1
        bdu64 = cons[:, 384:512]        # strict upper within 64-blocks, +1
        flow64 = cons[:, 512:640]       # lower outside 64-blocks, -1
        triuI = cons[:, 640:768]        # i<=j, +1
        ones = cons[:, 768:896]         # all ones

        sbias = persist.tile([33, 1], fp32, name="sbias", tag="sbias")
        nc.vector.memset(sbias[0:1, :], 1e-6 * DH)
        nc.vector.memset(sbias[32:33, :], 1e-6)
        ebias = persist.tile([128, 1], fp32, name="ebias", tag="ebias")
        nc.vector.memset(ebias[:], EPS)

        wot = []
        for p in range(4):
            t_ = persist.tile([128, D], f16, name=f"wo8_{p}", tag=f"wo8{p}")
            wot.append(t_)
        dg = []
        for i in range(2 * CT):
            t_ = persist.tile([128, CONV_K * 128], f16, name=f"dg{i}",
                              tag=f"dg{i}")
            dg.append(t_)
        cwvt = []
        for ct in range(CT):
            t_ = persist.tile([128, CONV_K], fp32, name=f"cwv{ct}",
                              tag=f"cwv{ct}")
            cwvt.append(t_)

        # qkv value tiles [128 chan, T] fp16 (nm: q=0, k=1, v=2)
        qkv = {}
        for nm in range(3):
            for ct in range(CT):
                qkv[(nm, ct)] = qkvp.tile([128, T], f16, name=f"qkv{nm}{ct}",
                                          tag=f"qkv{nm}{ct}")
        # oT: transposed outputs [128, T] fp16, one per column tile
        oT8 = [otp.tile([128, T], f16, name=f"oT8_{p}", tag=f"oT8{p}")
               for p in range(CT)]

        # ================= phase A =================
        with tc.tile_pool(name="rawp", bufs=1) as rawp, \
             tc.tile_pool(name="psA", bufs=1, space="PSUM") as psA, \
             tc.tile_pool(name="psC2", bufs=1, space="PSUM") as psC2, \
             tc.tile_pool(name="psL", bufs=1, space="PSUM") as psL, \
             tc.tile_pool(name="vtp", bufs=1) as vtp, \
             tc.tile_pool(name="l2p", bufs=1) as l2p:

            xt = []
            for p in range(KT8):
                t_ = rawp.tile([128, T], f16, name=f"x8_{p}", tag=f"x8{p}")
                xt.append(t_)
            wt = []
            for i in range(3 * KT8):
                t_ = rawp.tile([128, CG], f16, name=f"w8_{i}", tag=f"w8{i}")
                wt.append(t_)
            # DMA order matched to compute order: wq, x(nb0), wk, wv, x(nb1..)
            for i in range(KT8):
                nc.sync.dma_start(wt[i][:], w8_t[i])
            for p in range(KT8):
                nc.sync.dma_start(xt[p][:, 0:TOKB], x8_t[p][:, 0:TOKB])
            for i in range(KT8, 2 * KT8):
                nc.sync.dma_start(wt[i][:], w8_t[i])
            for p in range(KT8):
                nc.sync.dma_start(xt[p][:, TOKB:2 * TOKB],
                                  x8_t[p][:, TOKB:2 * TOKB])
            for i in range(2 * KT8, 3 * KT8):
                nc.sync.dma_start(wt[i][:], w8_t[i])
            nc.sync.dma_start(cons[:], consts.ap())
            for i in range(2 * CT):
                nc.sync.dma_start(dg[i][:], diag_t[i])
            for ct in range(CT):
                nc.sync.dma_start(cwvt[ct][:], cwv_t[ct])
            for nb in range(2, NB):
                for p in range(KT8):
                    sl = slice(nb * TOKB, (nb + 1) * TOKB)
                    nc.sync.dma_start(xt[p][:, sl], x8_t[p][:, sl])

            raw_prev = {}
            raw_all = {}

            def emit_proj(nb):
                tsl = slice(nb * TOKB, (nb + 1) * TOKB)
                raw_cur = {}
                for nm in range(3):
                    for ct in range(CT):
                        r = rawp.tile([128, PAD + TOKB], f16,
                                      name=f"raw{nm}{ct}{nb}",
                                      tag=f"raw{nm}{ct}", bufs=2)
                        raw_cur[(nm, ct)] = r
                        pp = psA.tile([128, TOKB], fp32,
                                      name=f"pp{nm}{ct}{nb}", tag="pp", bufs=3)
                        for p in range(KT8):
                            nc.tensor.matmul(
                                pp[:],
                                wt[nm * KT8 + p][:, ct * 128:(ct + 1) * 128],
                                xt[p][:, tsl], start=(p == 0),
                                stop=(p == KT8 - 1))
                        if (nm * CT + ct) % 3 == 0:
                            nc.scalar.copy(r[:, PAD:PAD + TOKB], pp[:])
                        else:
                            nc.vector.tensor_copy(r[:, PAD:PAD + TOKB], pp[:])
                        if nb == 0:
                            nc.vector.memset(r[:, 0:PAD], 0.0)
                        else:
                            nc.vector.tensor_copy(
                                r[:, 0:PAD],
                                raw_all[nb - 1][(nm, ct)][:, TOKB:TOKB + PAD])
                raw_all[nb] = raw_cur

            def emit_conv_l2(nb):
                tsl = slice(nb * TOKB, (nb + 1) * TOKB)
                raw_cur = raw_all[nb]
                for nm in range(3):
                    for ct in range(CT):
                        r = raw_cur[(nm, ct)]
                        dst = qkv[(nm, ct)][:, tsl]
                        if nm < 2:   # q/k: diagonal matmuls on PE
                            pc = psC2.tile([128, TOKB], fp32,
                                           name=f"pc{nm}{ct}{nb}", tag="pc",
                                           bufs=3)
                            dtile = dg[nm * CT + ct]
                            for tap in range(CONV_K):
                                nc.tensor.matmul(
                                    pc[:], dtile[:, tap * 128:(tap + 1) * 128],
                                    r[:, 1 + tap:1 + tap + TOKB],
                                    start=(tap == 0), stop=(tap == CONV_K - 1))
                            if SILU_NATIVE:
                                nc.scalar.activation(dst, pc[:], AF.Silu)
                            else:
                                sg = vtp.tile([128, TOKB], f16,
                                              name=f"sg{nm}{ct}{nb}", tag="sg",
                                              bufs=2)
                                nc.scalar.activation(sg[:], pc[:], AF.Sigmoid)
                                nc.vector.tensor_mul(dst, sg[:], pc[:])
                        else:        # v: DVE tree + Act silu
                            cw = cwvt[ct]
                            ve = nc.vector
                            tt_ = []
                            for tap in range(CONV_K):
                                tv = vtp.tile([128, TOKB], f16,
                                              name=f"vt{ct}{nb}{tap}",
                                              tag=f"vt{tap}", bufs=2)
                                ve.tensor_scalar_mul(
                                    tv[:], r[:, 1 + tap:1 + tap + TOKB],
                                    cw[:, tap:tap + 1])
                                tt_.append(tv)
                            ve.tensor_add(tt_[0][:], tt_[0][:], tt_[1][:])
                            ve.tensor_add(tt_[2][:], tt_[2][:], tt_[3][:])
                            ve.tensor_add(tt_[0][:], tt_[0][:], tt_[2][:])
                            if SILU_NATIVE:
                                nc.scalar.activation(dst, tt_[0][:], AF.Silu)
                            else:
                                sg = vtp.tile([128, TOKB], f16,
                                              name=f"sgv{ct}{nb}", tag="sg",
                                              bufs=2)
                                nc.scalar.activation(sg[:], tt_[0][:],
                                                     AF.Sigmoid)
                                nc.gpsimd.tensor_mul(dst, sg[:], tt_[0][:])
                # l2 norm per (head, nb), q and k together
                for head in range(2):
                    ct0 = 2 * head
                    sq = []
                    for nm in range(2):
                        for cth in range(2):
                            s_ = l2p.tile([128, TOKB], f16,
                                          name=f"sq{nm}{head}{cth}{nb}",
                                          tag=f"sq{nm}{cth}", bufs=2)
                            src = qkv[(nm, ct0 + cth)][:, tsl]
                            nc.gpsimd.tensor_mul(s_[:], src, src)
                            sq.append(s_)
                    prow = psL.tile([128, TOKB], fp32, name=f"pr{head}{nb}",
                                    tag="L", bufs=2)
                    for nm in range(2):
                        for cth in range(2):
                            nc.tensor.matmul(prow[nm * 32:nm * 32 + 1, :],
                                             ones[:, 0:1], sq[nm * 2 + cth][:],
                                             start=(cth == 0), stop=(cth == 1),
                                             tile_position=(0, nm * 32))
                    rowb = l2p.tile([33, TOKB], fp32, name=f"rb{head}{nb}",
                                    tag="rowb", bufs=2)
                    nc.scalar.activation(rowb[0:1, :], prow[0:1, :], AF.Sqrt,
                                         bias=sbias[0:1, :], scale=float(DH))
                    nc.scalar.activation(rowb[32:33, :], prow[32:33, :],
                                         AF.Sqrt, bias=sbias[32:33, :],
                                         scale=1.0)
                    rowh = l2p.tile([33, TOKB], f16, name=f"rh{head}{nb}",
                                    tag="rowh", bufs=2)
                    with nc.allow_low_precision(reason="rsqrt values O(1-30)"):
                        nc.vector.reciprocal(rowh[0:1, :], rowb[0:1, :])
                        nc.vector.reciprocal(rowh[32:33, :], rowb[32:33, :])
                    for nm in range(2):
                        pbc = psL.tile([128, TOKB], fp32,
                                       name=f"pbc{nm}{head}{nb}", tag="L",
                                       bufs=2)
                        nc.tensor.matmul(pbc[:],
                                         ones[nm * 32:nm * 32 + 1, 0:128],
                                         rowh[nm * 32:nm * 32 + 1, :],
                                         start=True, stop=True)
                        bcb = l2p.tile([128, TOKB], f16,
                                       name=f"bcb{nm}{head}{nb}", tag="bcb",
                                       bufs=2)
                        nc.vector.tensor_copy(bcb[:], pbc[:])
                        for cth in range(2):
                            dstq = qkv[(nm, ct0 + cth)][:, tsl]
                            nc.gpsimd.tensor_mul(dstq, dstq, bcb[:])

            emit_proj(0)
            for nb in range(1, NB):
                emit_proj(nb)
                emit_conv_l2(nb - 1)
            emit_conv_l2(NB - 1)

        if PHASE_LIMIT == 1:
            with tc.tile_pool(name="zf", bufs=1) as zf:
                z = zf.tile([128, D], f16, name="zt", tag="z")
                nc.vector.memset(z[:], 0.0)
                for tt in range(T // 128):
                    nc.sync.dma_start(out_t[tt], z[:])
            return

        # ================= phase B (+ phase C interleaved) =================
        # Stage-wavefront emission: precompute pipelines of GW chunk-units
        # advance stage-by-stage so every engine sees long runs of
        # independent work (engine queues are strictly in-order).
        with tc.tile_pool(name="bpre", bufs=1) as bpre, \
             tc.tile_pool(name="bchn", bufs=1) as bchn, \
             tc.tile_pool(name="psw", bufs=1, space="PSUM") as psw, \
             tc.tile_pool(name="outp", bufs=1) as outp:

            for p in range(4):
                nc.sync.dma_start(wot[p][:], wo8_t[p])

            s_sb = [None, None]
            st = {}          # (head, ch) -> per-unit state
            NBUF = 20

            def wtile(shape, dtyp, name):
                return psw.tile(shape, dtyp, name=name, tag="w", bufs=4)

            def ctile(shape, dtyp, name):
                return psw.tile(shape, dtyp, name=name, tag="c", bufs=4)

            def cp(eng, dst, src):
                if eng % 2 == 0:
                    nc.vector.tensor_copy(dst, src)
                else:
                    nc.scalar.copy(dst, src)

            def cpneg(eng, dst, src):
                if eng % 2 == 1:
                    nc.scalar.activation(dst, src, AF.Copy, scale=-1.0)
                else:
                    nc.vector.tensor_scalar_mul(dst, src, -1.0)

            def slices(head, ch):
                ct0 = 2 * head
                t0 = ch * C
                KT = [qkv[(1, ct0)][:, t0:t0 + C],
                      qkv[(1, ct0 + 1)][:, t0:t0 + C]]
                QT = [qkv[(0, ct0)][:, t0:t0 + C],
                      qkv[(0, ct0 + 1)][:, t0:t0 + C]]
                VT = [qkv[(2, ct0)][:, t0:t0 + C],
                      qkv[(2, ct0 + 1)][:, t0:t0 + C]]
                return KT, QT, VT

            # --- precompute stages; each stage(u) emits ops for one unit ---
            def s_pkv(u):
                head, ch = u
                KT, QT, VT = slices(head, ch)
                s = st[u]
                s["pkv"] = wtile([128, 512], f16, f"pkv{head}_{ch}")
                for i in range(2):
                    nc.tensor.transpose(s["pkv"][:, i * 128:(i + 1) * 128],
                                        KT[i], ident)
                    nc.tensor.transpose(
                        s["pkv"][:, 256 + i * 128:256 + (i + 1) * 128],
                        VT[i], ident)

            def s_kvcp(u, eng):
                head, ch = u
                s = st[u]
                s["kv"] = bpre.tile([128, 512], f16, name=f"kv{head}_{ch}",
                                    tag="kv", bufs=NBUF)
                cp(eng, s["kv"][:], s["pkv"][:])
                del s["pkv"]

            def s_pkk(u):
                head, ch = u
                KT, QT, VT = slices(head, ch)
                s = st[u]
                s["pkk"] = wtile([128, 128], fp32, f"pkk{head}_{ch}")
                for i in range(2):
                    nc.tensor.matmul(s["pkk"][:], KT[i], KT[i], start=(i == 0),
                                     stop=(i == 1))

            def s_masks(u, eng):
                head, ch = u
                s = st[u]
                kk = bpre.tile([128, 128], f16, name=f"kk{head}_{ch}",
                               tag="kk", bufs=NBUF)
                cp(eng, kk[:], s["pkk"][:])
                del s["pkk"]
                s["db"] = bpre.tile([128, 256], f16, name=f"db{head}_{ch}",
                                    tag="db", bufs=NBUF)
                s["flw"] = bpre.tile([128, 128], f16, name=f"flw{head}_{ch}",
                                     tag="flw", bufs=NBUF)
                nc.gpsimd.tensor_mul(s["db"][:, 0:128], kk[:], bdl64)
                nc.gpsimd.tensor_mul(s["db"][:, 128:256], kk[:], bdu64)
                nc.gpsimd.tensor_mul(s["flw"][:], kk[:], flow64)

            def s_px1(u):
                head, ch = u
                s = st[u]
                Bm, Dm = s["db"][:, 0:128], s["db"][:, 128:256]
                s["px1"] = wtile([128, 256], fp32, f"px1{head}_{ch}")
                nc.tensor.matmul(s["px1"][:, 0:128], Bm, Dm, start=True,
                                 stop=True)
                nc.tensor.matmul(s["px1"][:, 128:256], Dm, Bm, start=True,
                                 stop=True)

            def s_x1n(u, eng):
                head, ch = u
                s = st[u]
                s["x1n"] = bpre.tile([128, 256], f16, name=f"x1n{head}_{ch}",
                                     tag="x1n", bufs=NBUF)
                cpneg(eng, s["x1n"][:], s["px1"][:])
                del s["px1"]

            def s_pr1(u):
                head, ch = u
                s = st[u]
                Bm = s["db"][:, 0:128]
                X1n, X1tn = s["x1n"][:, 0:128], s["x1n"][:, 128:256]
                p = wtile([128, 256], fp32, f"pr1{head}_{ch}")
                s["pr1"] = p
                nc.tensor.matmul(p[:, 0:128], ident, ident, start=True,
                                 stop=False)
                nc.tensor.matmul(p[:, 0:128], identN, Bm, start=False,
                                 stop=False)
                nc.tensor.matmul(p[:, 0:128], X1n, identN, start=False,
                                 stop=False)
                nc.tensor.matmul(p[:, 0:128], X1n, Bm, start=False, stop=True)
                nc.tensor.matmul(p[:, 128:256], X1tn, X1n, start=True,
                                 stop=True)

            def s_r1x2(u, eng):
                head, ch = u
                s = st[u]
                s["r1x2"] = bpre.tile([128, 256], f16, name=f"r1x2{head}_{ch}",
                                      tag="r1x2", bufs=NBUF)
                cp(eng, s["r1x2"][:], s["pr1"][:])
                del s["pr1"]

            def s_pr2(u):
                head, ch = u
                s = st[u]
                X1n, X1tn = s["x1n"][:, 0:128], s["x1n"][:, 128:256]
                R1 = s["r1x2"][:, 0:128]
                X2 = s["r1x2"][:, 128:256]
                p = wtile([128, 256], fp32, f"pr2{head}_{ch}")
                s["pr2"] = p
                nc.tensor.matmul(p[:, 0:128], X1n, X1tn, start=True, stop=True)
                nc.tensor.matmul(p[:, 128:256], ident, R1, start=True,
                                 stop=False)
                nc.tensor.matmul(p[:, 128:256], X2, R1, start=False, stop=True)

            def s_x2r2(u, eng):
                head, ch = u
                s = st[u]
                s["x2r2"] = bpre.tile([128, 256], f16, name=f"x2r2{head}_{ch}",
                                      tag="x2r2", bufs=NBUF)
                cp(eng, s["x2r2"][:], s["pr2"][:])
                del s["pr2"], s["x1n"]

            def s_px4(u):
                head, ch = u
                s = st[u]
                X2t, R2 = s["x2r2"][:, 0:128], s["x2r2"][:, 128:256]
                X2 = s["r1x2"][:, 128:256]
                p = wtile([128, 128], fp32, f"px4{head}_{ch}")
                s["px4"] = p
                nc.tensor.matmul(p[:], X2t, X2, start=True, stop=True)

            def s_x4(u, eng):
                head, ch = u
                s = st[u]
                s["x4"] = bpre.tile([128, 128], f16, name=f"x4{head}_{ch}",
                                    tag="x4", bufs=NBUF)
                cp(eng, s["x4"][:], s["px4"][:])
                del s["px4"], s["r1x2"]

            def s_prm(u):
                head, ch = u
                s = st[u]
                R2 = s["x2r2"][:, 128:256]
                p = wtile([128, 128], fp32, f"prm{head}_{ch}")
                s["prm"] = p
                nc.tensor.matmul(p[:], ident, R2, start=True, stop=False)
                nc.tensor.matmul(p[:], s["x4"][:], R2, start=False, stop=True)

            def s_rm(u, eng):
                head, ch = u
                s = st[u]
                s["rm"] = bpre.tile([128, 128], f16, name=f"rm{head}_{ch}",
                                    tag="rm", bufs=NBUF)
                cp(eng, s["rm"][:], s["prm"][:])
                del s["prm"], s["x4"], s["x2r2"]

            def s_pxm0(u):
                head, ch = u
                s = st[u]
                s["pxma"] = wtile([64, 128], fp32, f"pxma{head}_{ch}")
                s["mt"] = bpre.tile([128, 128], f16, name=f"mt{head}_{ch}",
                                    tag="mt", bufs=NBUF)
                nc.tensor.matmul(s["pxma"][:], s["rm"][64:128, 64:128],
                                 ident[64:128, :], start=True, stop=True,
                                 tile_position=(64, 0))

            def s_msb0(u, eng):
                s = st[u]
                cp(eng, s["mt"][64:128, :], s["pxma"][:])
                del s["pxma"]

            def s_py(u):
                head, ch = u
                s = st[u]
                p = wtile([64, 128], fp32, f"py{head}_{ch}")
                s["py"] = p
                nc.tensor.matmul(p[:], s["flw"][64:128, 0:64],
                                 s["mt"][64:128, :], start=True, stop=True,
                                 tile_position=(64, 0))

            def s_ysb(u, eng):
                head, ch = u
                s = st[u]
                s["ysb"] = bpre.tile([64, 128], f16, name=f"y{head}_{ch}",
                                     tag="y", bufs=NBUF)
                nc.vector.tensor_add(s["ysb"][:], s["py"][:], ident[0:64, :])
                del s["py"], s["flw"]

            def s_pxm1(u):
                head, ch = u
                s = st[u]
                s["pxmb"] = wtile([64, 128], fp32, f"pxmb{head}_{ch}")
                nc.tensor.matmul(s["pxmb"][:], s["rm"][0:64, 0:64],
                                 s["ysb"][:], start=True, stop=True,
                                 tile_position=(0, 0))

            def s_msb1(u, eng):
                s = st[u]
                cp(eng, s["mt"][0:64, :], s["pxmb"][:])
                del s["pxmb"], s["ysb"], s["rm"]

            def s_pkq(u):
                head, ch = u
                KT, QT, VT = slices(head, ch)
                s = st[u]
                s["pkq"] = wtile([128, 128], fp32, f"pkq{head}_{ch}")
                for i in range(2):
                    nc.tensor.matmul(s["pkq"][:], KT[i], QT[i], start=(i == 0),
                                     stop=(i == 1))

            def s_pat(u, eng):
                head, ch = u
                s = st[u]
                s["pat"] = bpre.tile([128, 128], f16, name=f"pat{head}_{ch}",
                                     tag="pat", bufs=NBUF)
                nc.vector.tensor_mul(s["pat"][:], s["pkq"][:], triuI)
                del s["pkq"]

            def s_pkqk(u):
                s_pkq(u)
                s_pkk(u)

            def s_patmasks(u, eng):
                s_pat(u, eng)
                s_masks(u, eng)

            MM_STAGES = [s_pkv, s_pkqk, s_px1, s_pr1, s_pr2, s_px4,
                         s_prm, s_pxm0, s_py, s_pxm1]
            CP_STAGES = [s_kvcp, s_patmasks, s_x1n, s_r1x2, s_x2r2, s_x4,
                         s_rm, s_msb0, s_ysb, s_msb1]

            def precompute_wave(units):
                # interleave: mm-stage k over all units, then copy-stage k
                # (copy engine rotates per unit)
                for k in range(len(MM_STAGES)):
                    for i, u in enumerate(units):
                        MM_STAGES[k](u)
                    for i, u in enumerate(units):
                        CP_STAGES[k](u, (i + k) % 3)
                    yield k

            def chain_a(head, ch):
                KT, QT, VT = slices(head, ch)
                s = st[(head, ch)]
                vcd = s["kv"][:, 256:512]
                if ch == 0:
                    s["vks"] = vcd
                    return
                s_old = s_sb[head]
                pt = ctile([128, 256], fp32, f"pt{head}_{ch}")
                for j in range(2):
                    nc.tensor.matmul(pt[:], KT[j],
                                     s_old[:, j * 256:(j + 1) * 256],
                                     start=(j == 0), stop=(j == 1))
                vkst = bchn.tile([128, 256], f16, name=f"vks{head}_{ch}",
                                 tag="vks", bufs=3)
                nc.vector.tensor_sub(vkst[:], vcd, pt[:])
                s["vks"] = vkst[:]

            def chain_b(head, ch):
                s = st[(head, ch)]
                pu = ctile([128, 256], fp32, f"pu{head}_{ch}")
                nc.tensor.matmul(pu[:], s["mt"][:], s["vks"], start=True,
                                 stop=True)
                usb = bchn.tile([128, 256], f16, name=f"u{head}_{ch}", tag="u",
                                bufs=4)
                nc.scalar.copy(usb[:], pu[:])
                s["usb"] = usb

            def chain_c(head, ch):
                KT, QT, VT = slices(head, ch)
                s = st[(head, ch)]
                usb = s["usb"]
                kcd = s["kv"][:, 0:256]
                # S_new = S_old + K^T U'
                if ch < NCHUNK - 1:
                    ds = ctile([128, 512], fp32, f"ds{head}_{ch}")
                    for j in range(2):
                        nc.tensor.matmul(ds[:, j * 256:(j + 1) * 256],
                                         kcd[:, j * 128:(j + 1) * 128],
                                         usb[:], start=True, stop=True)
                    s_nb = bchn.tile([128, 512], f16, name=f"s{head}_{ch}",
                                     tag=f"s{head}", bufs=4)
                    if ch == 0:
                        nc.vector.tensor_copy(s_nb[:], ds[:])
                    else:
                        nc.vector.tensor_add(s_nb[:], s_sb[head][:], ds[:])
                    s_sb[head] = s_nb
                # O = Q S_old + Pat^T U'
                po = ctile([128, 256], fp32, f"po{head}_{ch}")
                if ch == 0:
                    nc.tensor.matmul(po[:], s["pat"][:], usb[:], start=True,
                                     stop=True)
                else:
                    s_old = s["s_old"]
                    for j in range(2):
                        nc.tensor.matmul(po[:], QT[j],
                                         s_old[:, j * 256:(j + 1) * 256],
                                         start=(j == 0), stop=False)
                    nc.tensor.matmul(po[:], s["pat"][:], usb[:], start=False,
                                     stop=True)
                s["po"] = po
                # RMS pipeline (off PE)
                osq = bchn.tile([128, 256], f16, name=f"osq{head}_{ch}",
                                tag="osq", bufs=3)
                ossq = bchn.tile([128, 1], fp32, name=f"ossq{head}_{ch}",
                                 tag="ossq", bufs=3)
                nc.scalar.activation(osq[:], po[:], AF.Square,
                                     accum_out=ossq[:])
                orsq = bchn.tile([128, 1], fp32, name=f"orsq{head}_{ch}",
                                 tag="orsq", bufs=3)
                nc.scalar.activation(orsq[:], ossq[:], AF.Sqrt,
                                     bias=ebias[:, 0:1], scale=1.0 / DH)
                nc.vector.reciprocal(orsq[:], orsq[:])
                onrm = bchn.tile([128, 256], f16, name=f"onrm{head}_{ch}",
                                 tag="onrm", bufs=3)
                nc.vector.tensor_scalar_mul(onrm[:], po[:], orsq[:])
                s["onrm"] = onrm

            def chain_d(head, ch):
                t0 = ch * C
                s = st.pop((head, ch))
                pot = ctile([128, 256], f16, f"pot{head}_{ch}")
                for i in range(2):
                    nc.tensor.transpose(pot[:, i * 128:(i + 1) * 128],
                                        s["onrm"][:, i * 128:(i + 1) * 128],
                                        ident)
                for i in range(2):
                    nc.scalar.copy(oT8[2 * head + i][:, t0:t0 + C],
                                   pot[:, i * 128:(i + 1) * 128])

            def chain_save_sold(head, ch):
                # stash the S the O-matmul needs (pre-update)
                if ch > 0:
                    st[(head, ch)]["s_old"] = s_sb[head]

            def phase_c(ch):
                t0 = ch * C
                if PHASE_LIMIT == 2:
                    for half in range(2):
                        of = outp.tile([128, 512], f16, name=f"of{ch}_{half}",
                                       tag="of", bufs=4)
                        nc.vector.memset(of[:], 0.0)
                        nc.sync.dma_start(
                            out_t[ch][:, half * 512:(half + 1) * 512], of[:])
                    return
                for half in range(2):
                    pf = ctile([128, 512], fp32, f"pf{ch}_{half}")
                    for p in range(CT):
                        nc.tensor.matmul(
                            pf[:], oT8[p][:, t0:t0 + C],
                            wot[p][:, half * 512:(half + 1) * 512],
                            start=(p == 0), stop=(p == CT - 1))
                    of = outp.tile([128, 512], f16, name=f"of{ch}_{half}",
                                   tag="of", bufs=4)
                    if half == 0:
                        nc.vector.tensor_copy(of[:], pf[:])
                    else:
                        nc.scalar.copy(of[:], pf[:])
                    nc.sync.dma_start(
                        out_t[ch][:, half * 512:(half + 1) * 512], of[:])

            # --- schedule: groups of GW chunks; chain parts of group g-1
            # spread across group g's wave stages (one hop per stage) ---
            GW = 4
            NGROUP = NCHUNK // GW
            for u in [(h, ch) for ch in range(NCHUNK) for h in range(2)]:
                st[u] = {}

            def make_tasks(g):
                tasks = {}          # stage -> list of thunks
                if g < 1 or PHASE_LIMIT == 3:
                    return tasks
                for i in range(GW):
                    ch = (g - 1) * GW + i
                    base = int(2.5 * i)
                    tasks.setdefault(base, []).extend([
                        lambda h=h, c=ch: (chain_save_sold(h, c),
                                           chain_a(h, c)) for h in range(2)])
                    tasks.setdefault(base + 1, []).extend([
                        lambda h=h, c=ch: chain_b(h, c) for h in range(2)])
                    tasks.setdefault(base + 2, []).extend([
                        lambda h=h, c=ch: chain_c(h, c) for h in range(2)])
                    tasks.setdefault(base + 3, []).extend([
                        lambda h=h, c=ch: chain_d(h, c) for h in range(2)])
                    tasks.setdefault(base + 4, []).append(
                        lambda c=ch: phase_c(c))
                return tasks

            def debug_dump():
                if dbg is None:
                    return
                dap = dbg.ap()
                idx = 0
                for nm in range(3):
                    for ct in range(CT):
                        tmp = outp.tile([128, T], fp32, name=f"dbg{nm}{ct}",
                                        tag="dbgt", bufs=1)
                        nc.vector.tensor_copy(tmp[:], qkv[(nm, ct)][:])
                        nc.sync.dma_start(dap[idx], tmp[:])
                        idx += 1
                for p in range(2):
                    tmp = outp.tile([128, T], fp32, name=f"dbgo{p}",
                                    tag="dbgt", bufs=1)
                    nc.vector.tensor_copy(tmp[:], oT8[2 * p][:])
                    nc.sync.dma_start(dap[idx], tmp[:])
                    idx += 1

            if PHASE_LIMIT == 3:
                z = outp.tile([128, D], f16, name="zt3", tag="of")
                nc.vector.memset(z[:], 0.0)
                for tt in range(T // 128):
                    nc.sync.dma_start(out_t[tt], z[:])
            for g in range(NGROUP + 1):
                tasks = make_tasks(g) if PHASE_LIMIT != 3 else {}
                if g < NGROUP:
                    units = [(h, g * GW + i) for i in range(GW)
                             for h in range(2)]
                    nstage = min(len(MM_STAGES), STAGE_LIMIT)
                    for k in range(nstage):
                        for t in tasks.pop(k, []):
                            t()
                        for u in units:
                            MM_STAGES[k](u)
                        for i, u in enumerate(units):
                            CP_STAGES[k](u, (i + k) % 2)
                    for k in sorted(tasks):
                        for t in tasks[k]:
                            t()
                else:
                    for k in sorted(tasks):
                        for t in tasks[k]:
                            t()
            debug_dump()


def _make_consts():
    ii = np.arange(128)
    blk = ii[:, None] // 64 == ii[None, :] // 64
    ident = np.eye(128, dtype=np.float32)
    bdl = ((ii[:, None] > ii[None, :]) & blk).astype(np.float32)
    bdu = ((ii[:, None] < ii[None, :]) & blk).astype(np.float32)
    flow = -((ii[:, None] > ii[None, :]) & ~blk).astype(np.float32)
    triuI = (ii[:, None] <= ii[None, :]).astype(np.float32)
    ones = np.ones((128, 128), np.float32)
    return np.concatenate([ident, -ident, bdl, bdu, flow, triuI, ones],
                          axis=1).astype(np.float16)


def _get_compiled():
    key = ("nc", SILU_NATIVE, DEBUG_DUMP, PHASE_LIMIT, STAGE_LIMIT)
    if key not in _CACHE:
        _CACHE[key] = _build_bass()
    return _CACHE[key]


def _make_in_maps(hidden_states, Wq, Wk, Wv, conv_wq, conv_wk, conv_wv,
                  onorm_w, Wo):
    hidden_states = np.asarray(hidden_states, np.float32)
    Wq = np.asarray(Wq, np.float32)
    Wk = np.asarray(Wk, np.float32)
    Wv = np.asarray(Wv, np.float32)
    Wo = np.asarray(Wo, np.float32)
    conv_wq = np.asarray(conv_wq, np.float32)
    conv_wk = np.asarray(conv_wk, np.float32)
    conv_wv = np.asarray(conv_wv, np.float32)
    onorm_w = np.asarray(onorm_w, np.float32)

    consts = _make_consts()
    Wo_eff = (Wo * np.tile(onorm_w, H)[:, None]).astype(np.float16)

    def pack_w(Wcols):  # [D, CG] -> [KT8, 128, CG] fp16
        return np.ascontiguousarray(
            Wcols.astype(np.float16).reshape(KT8, 128, CG))

    def pack_x(xT):     # [D, T] -> [KT8, 128, T] fp16
        return np.ascontiguousarray(
            xT.astype(np.float16).reshape(KT8, 128, T))

    def pack_wo(Wrows):  # [CG, D] -> [4, 128, D] fp16
        return np.ascontiguousarray(
            Wrows.astype(np.float16).reshape(4, 128, D))

    def make_diag(cw):   # [CG, K] -> [CT, 128, K*128] fp16 diag
        o = np.zeros((CT, 128, CONV_K * 128), np.float16)
        for ct in range(CT):
            for tap in range(CONV_K):
                np.fill_diagonal(o[ct, :, tap * 128:(tap + 1) * 128],
                                 cw[ct * 128:(ct + 1) * 128, tap
                                    ].astype(np.float16))
        return o

    in_maps = []
    for core in range(NCORES):
        b, g = divmod(core, 2)
        cols = slice(CG * g, CG * (g + 1))
        in_maps.append({
            "x8": pack_x(np.ascontiguousarray(hidden_states[b].T)),
            "w8": np.concatenate([pack_w(Wq[:, cols]), pack_w(Wk[:, cols]),
                                  pack_w(Wv[:, cols])], axis=0),
            "wo8": pack_wo(Wo_eff[cols, :]),
            "diag": np.concatenate([make_diag(conv_wq[cols]),
                                    make_diag(conv_wk[cols])], axis=0),
            "cwv": np.ascontiguousarray(
                conv_wv[cols].reshape(CT, 128, CONV_K)).astype(np.float32),
            "consts": consts,
        })
    return in_maps


def kernel(hidden_states, Wq, Wk, Wv, conv_wq, conv_wk, conv_wv, onorm_w, Wo):
    from concourse.bass_utils import run_bass_kernel_spmd

    in_maps = _make_in_maps(hidden_states, Wq, Wk, Wv, conv_wq, conv_wk,
                            conv_wv, onorm_w, Wo)
    nc = _get_compiled()
    res = run_bass_kernel_spmd(nc, in_maps, core_ids=list(range(NCORES)),
                               **_CACHE.get("run_kwargs", {}))
    _CACHE["last_results"] = res
    out = np.zeros((B, T, D), np.float32)
    for core in range(NCORES):
        out[core // 2] += res.results[core]["out"].astype(np.float32)
    return out


# revision 50
# speedup vs baseline: 1.0017x; 1.0017x over previous
"""DeltaNet forward kernel for 8 Trainium2 NeuronCores — v2 (restructured).

Problem (hardcoded): hidden_states [B=4, T=2048, D=1024], H=4 heads, Dh=256,
causal depthwise conv K=4 + silu on q/k/v projections, q/k l2-normalized per
head (q scaled Dh^-0.5), delta-rule recurrence over T, per-head RMSNorm,
merge heads, out = o @ Wo.

Sharding: core c -> batch c//2, head group c%2 (projection columns
[512*(c%2), 512*(c%2)+512)). Host sums the two row-parallel partials.

v2 design vs baseline:
- Projections and output matmul in fp8e4m3 + DoubleRow (0.5 cyc/row).
  Weights pre-scaled x64 host-side; 1/64 folded into conv weights / out copy.
- Depthwise conv as diagonal-matrix fp16 matmuls on PE for q/k (psum
  accumulate over shifted rhs views); v's conv on DVE (TSM 4x + TT-add tree).
- Phase B (delta recurrence, chunk C=128) in U-form: per-chunk S-independent
  precompute (parallel across all 32 head-chunks):
    KK = K K^T; 64-block masks D, B=D^T, FnT=-L^T;
    R^T = (I+B^8)(I+B^4)(I-B+B^2-B^3) masked doubling;
    M = (I+A)^{-1} via 2-block substitution on identity RHS;
    Mtn = -M^T; GTn_j = -K_j^T M^T; u0n = -M V; Pat = triu_incl(K Q^T).
  serial chain per head: U' = U0 - G S (3 mm); S += K^T U' (2 mm, persistent
  psum); O = Q S_old + Pat^T U' (3 mm) -> RMS -> transpose -> oT8 (fp8).
- Output projection fp8-DR from oT8, fp16 DMA out, f32 partial sum on host.
"""

import numpy as np

B, T, D = 4, 2048, 1024
H = 4
DH = D // H          # 256
CONV_K = 4
EPS = 1e-5
NCORES = 8
CG = 512             # columns per core (2 heads)
C = 128              # recurrence chunk length
NCHUNK = T // C      # 16
PAD = 4              # front zero padding on time axis for the causal conv
TOKB = 512           # token block
NB = T // TOKB       # 4
KP = D // 256        # 4 DR contraction pairs for D=1024
KT8 = D // 128       # 8 fp16 contraction tiles
CT = CG // 128       # 4 column tiles per core
WSCALE = 1.0         # no fp8 pre-scale in the fp16 path

_CACHE = {}
SILU_NATIVE = True  # CoreSim lacks Silu; set False for simulation runs
DEBUG_DUMP = False
PHASE_LIMIT = 0  # 1: only phase A, 2: A+B (no C); 0: full
STAGE_LIMIT = 99


def _build_bass():
    import concourse.bass as bass  # noqa: F401
    import concourse.bacc as bacc
    import concourse.mybir as mybir
    import concourse.tile as tile

    dt = mybir.dt
    nc = bacc.Bacc("TRN2", target_bir_lowering=False, debug=False)

    x8 = nc.dram_tensor("x8", [KT8, 128, T], dt.float16, kind="ExternalInput")
    w8 = nc.dram_tensor("w8", [3 * KT8, 128, CG], dt.float16,
                        kind="ExternalInput")
    wo8 = nc.dram_tensor("wo8", [4, 128, D], dt.float16,
                         kind="ExternalInput")
    diag = nc.dram_tensor("diag", [2 * CT, 128, CONV_K * 128], dt.float16,
                          kind="ExternalInput")
    cwv = nc.dram_tensor("cwv", [CT, 128, CONV_K], dt.float32,
                         kind="ExternalInput")
    consts = nc.dram_tensor("consts", [128, 7 * 128], dt.float16,
                            kind="ExternalInput")
    out = nc.dram_tensor("out", [T, D], dt.float16, kind="ExternalOutput")
    dbg = None
    if DEBUG_DUMP:
        dbg = nc.dram_tensor("dbg", [14, 128, T], dt.float32,
                             kind="ExternalOutput")

    with tile.TileContext(nc) as tc:
        _body(nc, tc, mybir, x8, w8, wo8, diag, cwv, consts, out, dbg)

    nc.compile()
    return nc


def _body(nc, tc, mybir, x8, w8, wo8, diag, cwv, consts, out, dbg=None):
    dt = mybir.dt
    AF = mybir.ActivationFunctionType
    PM = mybir.MatmulPerfMode.DoubleRow
    fp32 = dt.float32
    f16 = dt.float16
    f8 = dt.float8e4

    x8_t = x8.ap()          # [KP, 128, 2T]
    w8_t = w8.ap()          # [3KP, 128, 2CG]
    wo8_t = wo8.ap()        # [2, 128, 2D]
    diag_t = diag.ap()      # [2CT, 128, 4*128]
    cwv_t = cwv.ap()        # [CT, 128, 4]
    out_t = out.ap().rearrange("(n p) c -> n p c", p=128)   # [16,128,D]

    with tc.tile_pool(name="persist", bufs=1) as persist, \
         tc.tile_pool(name="qkvp", bufs=1) as qkvp, \
         tc.tile_pool(name="otp", bufs=1) as otp:

        # ---------------- constants / weights ----------------
        cons = persist.tile([128, 7 * 128], f16, name="cons", tag="cons")
        ident = cons[:, 0:128]
        identN = cons[:, 128:256]       # -I
        bdl64 = cons[:, 256:384]        # strict lower within 64-blocks, +1
        bdu64 = cons[:, 384:512]        # strict upper within 64-blocks, +1
        flow64 = cons[:, 512:640]       # lower outside 64-blocks, -1
        triuI = cons[:, 640:768]        # i<=j, +1
        ones = cons[:, 768:896]         # all ones

        sbias = persist.tile([33, 1], fp32, name="sbias", tag="sbias")
        nc.vector.memset(sbias[0:1, :], 1e-6 * DH)
        nc.vector.memset(sbias[32:33, :], 1e-6)
        ebias = persist.tile([128, 1], fp32, name="ebias", tag="ebias")
        nc.vector.memset(ebias[:], EPS)

        wot = []
        for p in range(4):
            t_ = persist.tile([128, D], f16, name=f"wo8_{p}", tag=f"wo8{p}")
            wot.append(t_)
        dg = []
        for i in range(2 * CT):
            t_ = persist.tile([128, CONV_K * 128], f16, name=f"dg{i}",
                              tag=f"dg{i}")
            dg.append(t_)
        cwvt = []
        for ct in range(CT):
            t_ = persist.tile([128, CONV_K], fp32, name=f"cwv{ct}",
                              tag=f"cwv{ct}")
            cwvt.append(t_)

        # qkv value tiles [128 chan, T] fp16 (nm: q=0, k=1, v=2)
        qkv = {}
        for nm in range(3):
            for ct in range(CT):
                qkv[(nm, ct)] = qkvp.tile([128, T], f16, name=f"qkv{nm}{ct}",
                                          tag=f"qkv{nm}{ct}")
        # oT: transposed outputs [128, T] fp16, one per column tile
        oT8 = [otp.tile([128, T], f16, name=f"oT8_{p}", tag=f"oT8{p}")
               for p in range(CT)]

        # ================= phase A =================
        with tc.tile_pool(name="rawp", bufs=1) as rawp, \
             tc.tile_pool(name="psA", bufs=1, space="PSUM") as psA, \
             tc.tile_pool(name="psC2", bufs=1, space="PSUM") as psC2, \
             tc.tile_pool(name="psL", bufs=1, space="PSUM") as psL, \
             tc.tile_pool(name="vtp", bufs=1) as vtp, \
             tc.tile_pool(name="l2p", bufs=1) as l2p:

            xt = []
            for p in range(KT8):
                t_ = rawp.tile([128, T], f16, name=f"x8_{p}", tag=f"x8{p}")
                xt.append(t_)
            wt = []
            for i in range(3 * KT8):
                t_ = rawp.tile([128, CG], f16, name=f"w8_{i}", tag=f"w8{i}")
                wt.append(t_)
            # DMA order matched to compute order: wq, x(nb0), wk, wv, x(nb1..)
            for p in range(KT8):
                nc.scalar.dma_start(xt[p][:, 0:TOKB], x8_t[p][:, 0:TOKB])
            for i in range(KT8):
                nc.sync.dma_start(wt[i][:], w8_t[i])
            for i in range(KT8, 2 * KT8):
                nc.sync.dma_start(wt[i][:], w8_t[i])
            for p in range(KT8):
                nc.sync.dma_start(xt[p][:, TOKB:2 * TOKB],
                                  x8_t[p][:, TOKB:2 * TOKB])
            for i in range(2 * KT8, 3 * KT8):
                nc.sync.dma_start(wt[i][:], w8_t[i])
            nc.sync.dma_start(cons[:], consts.ap())
            for i in range(2 * CT):
                nc.sync.dma_start(dg[i][:], diag_t[i])
            for ct in range(CT):
                nc.sync.dma_start(cwvt[ct][:], cwv_t[ct])
            for nb in range(2, NB):
                for p in range(KT8):
                    sl = slice(nb * TOKB, (nb + 1) * TOKB)
                    nc.sync.dma_start(xt[p][:, sl], x8_t[p][:, sl])

            raw_prev = {}
            raw_all = {}

            def emit_proj(nb):
                tsl = slice(nb * TOKB, (nb + 1) * TOKB)
                raw_cur = {}
                for nm in range(3):
                    for ct in range(CT):
                        r = rawp.tile([128, PAD + TOKB], f16,
                                      name=f"raw{nm}{ct}{nb}",
                                      tag=f"raw{nm}{ct}", bufs=2)
                        raw_cur[(nm, ct)] = r
                        pp = psA.tile([128, TOKB], fp32,
                                      name=f"pp{nm}{ct}{nb}", tag="pp", bufs=3)
                        for p in range(KT8):
                            nc.tensor.matmul(
                                pp[:],
                                wt[nm * KT8 + p][:, ct * 128:(ct + 1) * 128],
                                xt[p][:, tsl], start=(p == 0),
                                stop=(p == KT8 - 1))
                        if (nm * CT + ct) % 3 == 0:
                            nc.scalar.copy(r[:, PAD:PAD + TOKB], pp[:])
                        else:
                            nc.vector.tensor_copy(r[:, PAD:PAD + TOKB], pp[:])
                        if nb == 0:
                            nc.vector.memset(r[:, 0:PAD], 0.0)
                        else:
                            nc.vector.tensor_copy(
                                r[:, 0:PAD],
                                raw_all[nb - 1][(nm, ct)][:, TOKB:TOKB + PAD])
                raw_all[nb] = raw_cur

            def emit_conv_l2(nb):
                tsl = slice(nb * TOKB, (nb + 1) * TOKB)
                raw_cur = raw_all[nb]
                for nm in range(3):
                    for ct in range(CT):
                        r = raw_cur[(nm, ct)]
                        dst = qkv[(nm, ct)][:, tsl]
                        if nm < 2:   # q/k: diagonal matmuls on PE
                            pc = psC2.tile([128, TOKB], fp32,
                                           name=f"pc{nm}{ct}{nb}", tag="pc",
                                           bufs=3)
                            dtile = dg[nm * CT + ct]
                            for tap in range(CONV_K):
                                nc.tensor.matmul(
                                    pc[:], dtile[:, tap * 128:(tap + 1) * 128],
                                    r[:, 1 + tap:1 + tap + TOKB],
                                    start=(tap == 0), stop=(tap == CONV_K - 1))
                            if SILU_NATIVE:
                                nc.scalar.activation(dst, pc[:], AF.Silu)
                            else:
                                sg = vtp.tile([128, TOKB], f16,
                                              name=f"sg{nm}{ct}{nb}", tag="sg",
                                              bufs=2)
                                nc.scalar.activation(sg[:], pc[:], AF.Sigmoid)
                                nc.vector.tensor_mul(dst, sg[:], pc[:])
                        else:        # v: DVE tree + Act silu
                            cw = cwvt[ct]
                            ve = nc.vector
                            tt_ = []
                            for tap in range(CONV_K):
                                tv = vtp.tile([128, TOKB], f16,
                                              name=f"vt{ct}{nb}{tap}",
                                              tag=f"vt{tap}", bufs=2)
                                ve.tensor_scalar_mul(
                                    tv[:], r[:, 1 + tap:1 + tap + TOKB],
                                    cw[:, tap:tap + 1])
                                tt_.append(tv)
                            ve.tensor_add(tt_[0][:], tt_[0][:], tt_[1][:])
                            ve.tensor_add(tt_[2][:], tt_[2][:], tt_[3][:])
                            ve.tensor_add(tt_[0][:], tt_[0][:], tt_[2][:])
                            if SILU_NATIVE:
                                nc.scalar.activation(dst, tt_[0][:], AF.Silu)
                            else:
                                sg = vtp.tile([128, TOKB], f16,
                                              name=f"sgv{ct}{nb}", tag="sg",
                                              bufs=2)
                                nc.scalar.activation(sg[:], tt_[0][:],
                                                     AF.Sigmoid)
                                nc.gpsimd.tensor_mul(dst, sg[:], tt_[0][:])
                # l2 norm per (head, nb), q and k together
                for head in range(2):
                    ct0 = 2 * head
                    sq = []
                    for nm in range(2):
                        for cth in range(2):
                            s_ = l2p.tile([128, TOKB], f16,
                                          name=f"sq{nm}{head}{cth}{nb}",
                                          tag=f"sq{nm}{cth}", bufs=2)
                            src = qkv[(nm, ct0 + cth)][:, tsl]
                            nc.gpsimd.tensor_mul(s_[:], src, src)
                            sq.append(s_)
                    prow = psL.tile([128, TOKB], fp32, name=f"pr{head}{nb}",
                                    tag="L", bufs=2)
                    for nm in range(2):
                        for cth in range(2):
                            nc.tensor.matmul(prow[nm * 32:nm * 32 + 1, :],
                                             ones[:, 0:1], sq[nm * 2 + cth][:],
                                             start=(cth == 0), stop=(cth == 1),
                                             tile_position=(0, nm * 32))
                    rowb = l2p.tile([33, TOKB], fp32, name=f"rb{head}{nb}",
                                    tag="rowb", bufs=2)
                    nc.scalar.activation(rowb[0:1, :], prow[0:1, :], AF.Sqrt,
                                         bias=sbias[0:1, :], scale=float(DH))
                    nc.scalar.activation(rowb[32:33, :], prow[32:33, :],
                                         AF.Sqrt, bias=sbias[32:33, :],
                                         scale=1.0)
                    rowh = l2p.tile([33, TOKB], f16, name=f"rh{head}{nb}",
                                    tag="rowh", bufs=2)
                    with nc.allow_low_precision(reason="rsqrt values O(1-30)"):
                        nc.vector.reciprocal(rowh[0:1, :], rowb[0:1, :])
                        nc.vector.reciprocal(rowh[32:33, :], rowb[32:33, :])
                    for nm in range(2):
                        pbc = psL.tile([128, TOKB], fp32,
                                       name=f"pbc{nm}{head}{nb}", tag="L",
                                       bufs=2)
                        nc.tensor.matmul(pbc[:],
                                         ones[nm * 32:nm * 32 + 1, 0:128],
                                         rowh[nm * 32:nm * 32 + 1, :],
                                         start=True, stop=True)
                        bcb = l2p.tile([128, TOKB], f16,
                                       name=f"bcb{nm}{head}{nb}", tag="bcb",
                                       bufs=2)
                        nc.vector.tensor_copy(bcb[:], pbc[:])
                        for cth in range(2):
                            dstq = qkv[(nm, ct0 + cth)][:, tsl]
                            nc.gpsimd.tensor_mul(dstq, dstq, bcb[:])

            emit_proj(0)
            for nb in range(1, NB):
                emit_proj(nb)
                emit_conv_l2(nb - 1)
            emit_conv_l2(NB - 1)

        if PHASE_LIMIT == 1:
            with tc.tile_pool(name="zf", bufs=1) as zf:
                z = zf.tile([128, D], f16, name="zt", tag="z")
                nc.vector.memset(z[:], 0.0)
                for tt in range(T // 128):
                    nc.sync.dma_start(out_t[tt], z[:])
            return

        # ================= phase B (+ phase C interleaved) =================
        # Stage-wavefront emission: precompute pipelines of GW chunk-units
        # advance stage-by-stage so every engine sees long runs of
        # independent work (engine queues are strictly in-order).
        with tc.tile_pool(name="bpre", bufs=1) as bpre, \
             tc.tile_pool(name="bchn", bufs=1) as bchn, \
             tc.tile_pool(name="psw", bufs=1, space="PSUM") as psw, \
             tc.tile_pool(name="outp", bufs=1) as outp:

            for p in range(4):
                nc.sync.dma_start(wot[p][:], wo8_t[p])

            s_sb = [None, None]
            st = {}          # (head, ch) -> per-unit state
            NBUF = 20

            def wtile(shape, dtyp, name):
                return psw.tile(shape, dtyp, name=name, tag="w", bufs=4)

            def ctile(shape, dtyp, name):
                return psw.tile(shape, dtyp, name=name, tag="c", bufs=4)

            def cp(eng, dst, src):
                if eng % 2 == 0:
                    nc.vector.tensor_copy(dst, src)
                else:
                    nc.scalar.copy(dst, src)

            def cpneg(eng, dst, src):
                if eng % 2 == 1:
                    nc.scalar.activation(dst, src, AF.Copy, scale=-1.0)
                else:
                    nc.vector.tensor_scalar_mul(dst, src, -1.0)

            def slices(head, ch):
                ct0 = 2 * head
                t0 = ch * C
                KT = [qkv[(1, ct0)][:, t0:t0 + C],
                      qkv[(1, ct0 + 1)][:, t0:t0 + C]]
                QT = [qkv[(0, ct0)][:, t0:t0 + C],
                      qkv[(0, ct0 + 1)][:, t0:t0 + C]]
                VT = [qkv[(2, ct0)][:, t0:t0 + C],
                      qkv[(2, ct0 + 1)][:, t0:t0 + C]]
                return KT, QT, VT

            # --- precompute stages; each stage(u) emits ops for one unit ---
            def s_pkv(u):
                head, ch = u
                KT, QT, VT = slices(head, ch)
                s = st[u]
                s["pkv"] = wtile([128, 512], f16, f"pkv{head}_{ch}")
                for i in range(2):
                    nc.tensor.transpose(s["pkv"][:, i * 128:(i + 1) * 128],
                                        KT[i], ident)
                    nc.tensor.transpose(
                        s["pkv"][:, 256 + i * 128:256 + (i + 1) * 128],
                        VT[i], ident)

            def s_kvcp(u, eng):
                head, ch = u
                s = st[u]
                s["kv"] = bpre.tile([128, 512], f16, name=f"kv{head}_{ch}",
                                    tag="kv", bufs=NBUF)
                cp(eng, s["kv"][:], s["pkv"][:])
                del s["pkv"]

            def s_pkk(u):
                head, ch = u
                KT, QT, VT = slices(head, ch)
                s = st[u]
                s["pkk"] = wtile([128, 128], fp32, f"pkk{head}_{ch}")
                for i in range(2):
                    nc.tensor.matmul(s["pkk"][:], KT[i], KT[i], start=(i == 0),
                                     stop=(i == 1))

            def s_masks(u, eng):
                head, ch = u
                s = st[u]
                kk = bpre.tile([128, 128], f16, name=f"kk{head}_{ch}",
                               tag="kk", bufs=NBUF)
                cp(eng, kk[:], s["pkk"][:])
                del s["pkk"]
                s["db"] = bpre.tile([128, 256], f16, name=f"db{head}_{ch}",
                                    tag="db", bufs=NBUF)
                s["flw"] = bpre.tile([128, 128], f16, name=f"flw{head}_{ch}",
                                     tag="flw", bufs=NBUF)
                nc.gpsimd.tensor_mul(s["db"][:, 0:128], kk[:], bdl64)
                nc.gpsimd.tensor_mul(s["db"][:, 128:256], kk[:], bdu64)
                nc.gpsimd.tensor_mul(s["flw"][:], kk[:], flow64)

            def s_px1(u):
                head, ch = u
                s = st[u]
                Bm, Dm = s["db"][:, 0:128], s["db"][:, 128:256]
                s["px1"] = wtile([128, 256], fp32, f"px1{head}_{ch}")
                nc.tensor.matmul(s["px1"][:, 0:128], Bm, Dm, start=True,
                                 stop=True)
                nc.tensor.matmul(s["px1"][:, 128:256], Dm, Bm, start=True,
                                 stop=True)

            def s_x1n(u, eng):
                head, ch = u
                s = st[u]
                s["x1n"] = bpre.tile([128, 256], f16, name=f"x1n{head}_{ch}",
                                     tag="x1n", bufs=NBUF)
                cpneg(eng, s["x1n"][:], s["px1"][:])
                del s["px1"]

            def s_pr1(u):
                head, ch = u
                s = st[u]
                Bm = s["db"][:, 0:128]
                X1n, X1tn = s["x1n"][:, 0:128], s["x1n"][:, 128:256]
                p = wtile([128, 256], fp32, f"pr1{head}_{ch}")
                s["pr1"] = p
                nc.tensor.matmul(p[:, 0:128], ident, ident, start=True,
                                 stop=False)
                nc.tensor.matmul(p[:, 0:128], identN, Bm, start=False,
                                 stop=False)
                nc.tensor.matmul(p[:, 0:128], X1n, identN, start=False,
                                 stop=False)
                nc.tensor.matmul(p[:, 0:128], X1n, Bm, start=False, stop=True)
                nc.tensor.matmul(p[:, 128:256], X1tn, X1n, start=True,
                                 stop=True)

            def s_r1x2(u, eng):
                head, ch = u
                s = st[u]
                s["r1x2"] = bpre.tile([128, 256], f16, name=f"r1x2{head}_{ch}",
                                      tag="r1x2", bufs=NBUF)
                cp(eng, s["r1x2"][:], s["pr1"][:])
                del s["pr1"]

            def s_pr2(u):
                head, ch = u
                s = st[u]
                X1n, X1tn = s["x1n"][:, 0:128], s["x1n"][:, 128:256]
                R1 = s["r1x2"][:, 0:128]
                X2 = s["r1x2"][:, 128:256]
                p = wtile([128, 256], fp32, f"pr2{head}_{ch}")
                s["pr2"] = p
                nc.tensor.matmul(p[:, 0:128], X1n, X1tn, start=True, stop=True)
                nc.tensor.matmul(p[:, 128:256], ident, R1, start=True,
                                 stop=False)
                nc.tensor.matmul(p[:, 128:256], X2, R1, start=False, stop=True)

            def s_x2r2(u, eng):
                head, ch = u
                s = st[u]
                s["x2r2"] = bpre.tile([128, 256], f16, name=f"x2r2{head}_{ch}",
                                      tag="x2r2", bufs=NBUF)
                cp(eng, s["x2r2"][:], s["pr2"][:])
                del s["pr2"], s["x1n"]

            def s_px4(u):
                head, ch = u
                s = st[u]
                X2t, R2 = s["x2r2"][:, 0:128], s["x2r2"][:, 128:256]
                X2 = s["r1x2"][:, 128:256]
                p = wtile([128, 128], fp32, f"px4{head}_{ch}")
                s["px4"] = p
                nc.tensor.matmul(p[:], X2t, X2, start=True, stop=True)

            def s_x4(u, eng):
                head, ch = u
                s = st[u]
                s["x4"] = bpre.tile([128, 128], f16, name=f"x4{head}_{ch}",
                                    tag="x4", bufs=NBUF)
                cp(eng, s["x4"][:], s["px4"][:])
                del s["px4"], s["r1x2"]

            def s_prm(u):
                head, ch = u
                s = st[u]
                R2 = s["x2r2"][:, 128:256]
                p = wtile([128, 128], fp32, f"prm{head}_{ch}")
                s["prm"] = p
                nc.tensor.matmul(p[:], ident, R2, start=True, stop=False)
                nc.tensor.matmul(p[:], s["x4"][:], R2, start=False, stop=True)

            def s_rm(u, eng):
                head, ch = u
                s = st[u]
                s["rm"] = bpre.tile([128, 128], f16, name=f"rm{head}_{ch}",
                                    tag="rm", bufs=NBUF)
                cp(eng, s["rm"][:], s["prm"][:])
                del s["prm"], s["x4"], s["x2r2"]

            def s_pxm0(u):
                head, ch = u
                s = st[u]
                s["pxma"] = wtile([64, 128], fp32, f"pxma{head}_{ch}")
                s["mt"] = bpre.tile([128, 128], f16, name=f"mt{head}_{ch}",
                                    tag="mt", bufs=NBUF)
                nc.tensor.matmul(s["pxma"][:], s["rm"][64:128, 64:128],
                                 ident[64:128, :], start=True, stop=True,
                                 tile_position=(64, 0))

            def s_msb0(u, eng):
                s = st[u]
                cp(eng, s["mt"][64:128, :], s["pxma"][:])
                del s["pxma"]

            def s_py(u):
                head, ch = u
                s = st[u]
                p = wtile([64, 128], fp32, f"py{head}_{ch}")
                s["py"] = p
                nc.tensor.matmul(p[:], s["flw"][64:128, 0:64],
                                 s["mt"][64:128, :], start=True, stop=True,
                                 tile_position=(64, 0))

            def s_ysb(u, eng):
                head, ch = u
                s = st[u]
                s["ysb"] = bpre.tile([64, 128], f16, name=f"y{head}_{ch}",
                                     tag="y", bufs=NBUF)
                nc.vector.tensor_add(s["ysb"][:], s["py"][:], ident[0:64, :])
                del s["py"], s["flw"]

            def s_pxm1(u):
                head, ch = u
                s = st[u]
                s["pxmb"] = wtile([64, 128], fp32, f"pxmb{head}_{ch}")
                nc.tensor.matmul(s["pxmb"][:], s["rm"][0:64, 0:64],
                                 s["ysb"][:], start=True, stop=True,
                                 tile_position=(0, 0))

            def s_msb1(u, eng):
                s = st[u]
                cp(eng, s["mt"][0:64, :], s["pxmb"][:])
                del s["pxmb"], s["ysb"], s["rm"]

            def s_pkq(u):
                head, ch = u
                KT, QT, VT = slices(head, ch)
                s = st[u]
                s["pkq"] = wtile([128, 128], fp32, f"pkq{head}_{ch}")
                for i in range(2):
                    nc.tensor.matmul(s["pkq"][:], KT[i], QT[i], start=(i == 0),
                                     stop=(i == 1))

            def s_pat(u, eng):
                head, ch = u
                s = st[u]
                s["pat"] = bpre.tile([128, 128], f16, name=f"pat{head}_{ch}",
                                     tag="pat", bufs=NBUF)
                nc.vector.tensor_mul(s["pat"][:], s["pkq"][:], triuI)
                del s["pkq"]

            def s_pkqk(u):
                s_pkq(u)
                s_pkk(u)

            def s_patmasks(u, eng):
                s_pat(u, eng)
                s_masks(u, eng)

            MM_STAGES = [s_pkv, s_pkqk, s_px1, s_pr1, s_pr2, s_px4,
                         s_prm, s_pxm0, s_py, s_pxm1]
            CP_STAGES = [s_kvcp, s_patmasks, s_x1n, s_r1x2, s_x2r2, s_x4,
                         s_rm, s_msb0, s_ysb, s_msb1]

            def precompute_wave(units):
                # interleave: mm-stage k over all units, then copy-stage k
                # (copy engine rotates per unit)
                for k in range(len(MM_STAGES)):
                    for i, u in enumerate(units):
                        MM_STAGES[k](u)
                    for i, u in enumerate(units):
                        CP_STAGES[k](u, (i + k) % 3)
                    yield k

            def chain_a(head, ch):
                KT, QT, VT = slices(head, ch)
                s = st[(head, ch)]
                vcd = s["kv"][:, 256:512]
                if ch == 0:
                    s["vks"] = vcd
                    return
                s_old = s_sb[head]
                pt = ctile([128, 256], fp32, f"pt{head}_{ch}")
                for j in range(2):
                    nc.tensor.matmul(pt[:], KT[j],
                                     s_old[:, j * 256:(j + 1) * 256],
                                     start=(j == 0), stop=(j == 1))
                vkst = bchn.tile([128, 256], f16, name=f"vks{head}_{ch}",
                                 tag="vks", bufs=3)
                nc.vector.tensor_sub(vkst[:], vcd, pt[:])
                s["vks"] = vkst[:]

            def chain_b(head, ch):
                s = st[(head, ch)]
                pu = ctile([128, 256], fp32, f"pu{head}_{ch}")
                nc.tensor.matmul(pu[:], s["mt"][:], s["vks"], start=True,
                                 stop=True)
                usb = bchn.tile([128, 256], f16, name=f"u{head}_{ch}", tag="u",
                                bufs=4)
                nc.scalar.copy(usb[:], pu[:])
                s["usb"] = usb

            def chain_c(head, ch):
                KT, QT, VT = slices(head, ch)
                s = st[(head, ch)]
                usb = s["usb"]
                kcd = s["kv"][:, 0:256]
                # S_new = S_old + K^T U'
                if ch < NCHUNK - 1:
                    ds = ctile([128, 512], fp32, f"ds{head}_{ch}")
                    for j in range(2):
                        nc.tensor.matmul(ds[:, j * 256:(j + 1) * 256],
                                         kcd[:, j * 128:(j + 1) * 128],
                                         usb[:], start=True, stop=True)
                    s_nb = bchn.tile([128, 512], f16, name=f"s{head}_{ch}",
                                     tag=f"s{head}", bufs=4)
                    if ch == 0:
                        nc.vector.tensor_copy(s_nb[:], ds[:])
                    else:
                        nc.vector.tensor_add(s_nb[:], s_sb[head][:], ds[:])
                    s_sb[head] = s_nb
                # O = Q S_old + Pat^T U'
                po = ctile([128, 256], fp32, f"po{head}_{ch}")
                if ch == 0:
                    nc.tensor.matmul(po[:], s["pat"][:], usb[:], start=True,
                                     stop=True)
                else:
                    s_old = s["s_old"]
                    for j in range(2):
                        nc.tensor.matmul(po[:], QT[j],
                                         s_old[:, j * 256:(j + 1) * 256],
                                         start=(j == 0), stop=False)
                    nc.tensor.matmul(po[:], s["pat"][:], usb[:], start=False,
                                     stop=True)
                s["po"] = po
                # RMS pipeline (off PE)
                osq = bchn.tile([128, 256], f16, name=f"osq{head}_{ch}",
                                tag="osq", bufs=3)
                ossq = bchn.tile([128, 1], fp32, name=f"ossq{head}_{ch}",
                                 tag="ossq", bufs=3)
                nc.scalar.activation(osq[:], po[:], AF.Square,
                                     accum_out=ossq[:])
                orsq = bchn.tile([128, 1], fp32, name=f"orsq{head}_{ch}",
                                 tag="orsq", bufs=3)
                nc.scalar.activation(orsq[:], ossq[:], AF.Sqrt,
                                     bias=ebias[:, 0:1], scale=1.0 / DH)
                nc.vector.reciprocal(orsq[:], orsq[:])
                onrm = bchn.tile([128, 256], f16, name=f"onrm{head}_{ch}",
                                 tag="onrm", bufs=3)
                nc.vector.tensor_scalar_mul(onrm[:], po[:], orsq[:])
                s["onrm"] = onrm

            def chain_d(head, ch):
                t0 = ch * C
                s = st.pop((head, ch))
                pot = ctile([128, 256], f16, f"pot{head}_{ch}")
                for i in range(2):
                    nc.tensor.transpose(pot[:, i * 128:(i + 1) * 128],
                                        s["onrm"][:, i * 128:(i + 1) * 128],
                                        ident)
                for i in range(2):
                    nc.scalar.copy(oT8[2 * head + i][:, t0:t0 + C],
                                   pot[:, i * 128:(i + 1) * 128])

            def chain_save_sold(head, ch):
                # stash the S the O-matmul needs (pre-update)
                if ch > 0:
                    st[(head, ch)]["s_old"] = s_sb[head]

            def phase_c(ch):
                t0 = ch * C
                if PHASE_LIMIT == 2:
                    for half in range(2):
                        of = outp.tile([128, 512], f16, name=f"of{ch}_{half}",
                                       tag="of", bufs=4)
                        nc.vector.memset(of[:], 0.0)
                        nc.sync.dma_start(
                            out_t[ch][:, half * 512:(half + 1) * 512], of[:])
                    return
                for half in range(2):
                    pf = ctile([128, 512], fp32, f"pf{ch}_{half}")
                    for p in range(CT):
                        nc.tensor.matmul(
                            pf[:], oT8[p][:, t0:t0 + C],
                            wot[p][:, half * 512:(half + 1) * 512],
                            start=(p == 0), stop=(p == CT - 1))
                    of = outp.tile([128, 512], f16, name=f"of{ch}_{half}",
                                   tag="of", bufs=4)
                    if half == 0:
                        nc.vector.tensor_copy(of[:], pf[:])
                    else:
                        nc.scalar.copy(of[:], pf[:])
                    nc.sync.dma_start(
                        out_t[ch][:, half * 512:(half + 1) * 512], of[:])

            # --- schedule: groups of GW chunks; chain parts of group g-1
            # spread across group g's wave stages (one hop per stage) ---
            GW = 4
            NGROUP = NCHUNK // GW
            for u in [(h, ch) for ch in range(NCHUNK) for h in range(2)]:
                st[u] = {}

            def make_tasks(g):
                tasks = {}          # stage -> list of thunks
                if g < 1 or PHASE_LIMIT == 3:
                    return tasks
                for i in range(GW):
                    ch = (g - 1) * GW + i
                    base = int(2.5 * i)
                    tasks.setdefault(base, []).extend([
                        lambda h=h, c=ch: (chain_save_sold(h, c),
                                           chain_a(h, c)) for h in range(2)])
                    tasks.setdefault(base + 1, []).extend([
                        lambda h=h, c=ch: chain_b(h, c) for h in range(2)])
                    tasks.setdefault(base + 2, []).extend([
                        lambda h=h, c=ch: chain_c(h, c) for h in range(2)])
                    tasks.setdefault(base + 3, []).extend([
                        lambda h=h, c=ch: chain_d(h, c) for h in range(2)])
                    tasks.setdefault(base + 4, []).append(
                        lambda c=ch: phase_c(c))
                return tasks

            def debug_dump():
                if dbg is None:
                    return
                dap = dbg.ap()
                idx = 0
                for nm in range(3):
                    for ct in range(CT):
                        tmp = outp.tile([128, T], fp32, name=f"dbg{nm}{ct}",
                                        tag="dbgt", bufs=1)
                        nc.vector.tensor_copy(tmp[:], qkv[(nm, ct)][:])
                        nc.sync.dma_start(dap[idx], tmp[:])
                        idx += 1
                for p in range(2):
                    tmp = outp.tile([128, T], fp32, name=f"dbgo{p}",
                                    tag="dbgt", bufs=1)
                    nc.vector.tensor_copy(tmp[:], oT8[2 * p][:])
                    nc.sync.dma_start(dap[idx], tmp[:])
                    idx += 1

            if PHASE_LIMIT == 3:
                z = outp.tile([128, D], f16, name="zt3", tag="of")
                nc.vector.memset(z[:], 0.0)
                for tt in range(T // 128):
                    nc.sync.dma_start(out_t[tt], z[:])
            for g in range(NGROUP + 1):
                tasks = make_tasks(g) if PHASE_LIMIT != 3 else {}
                if g < NGROUP:
                    units = [(h, g * GW + i) for i in range(GW)
                             for h in range(2)]
                    nstage = min(len(MM_STAGES), STAGE_LIMIT)
                    for k in range(nstage):
                        for t in tasks.pop(k, []):
                            t()
                        for u in units:
                            MM_STAGES[k](u)
                        for i, u in enumerate(units):
                            CP_STAGES[k](u, (i + k) % 2)
                    for k in sorted(tasks):
                        for t in tasks[k]:
                            t()
                else:
                    for k in sorted(tasks):
                        for t in tasks[k]:
                            t()
            debug_dump()


def _make_consts():
    ii = np.arange(128)
    blk = ii[:, None] // 64 == ii[None, :] // 64
    ident = np.eye(128, dtype=np.float32)
    bdl = ((ii[:, None] > ii[None, :]) & blk).astype(np.float32)
    bdu = ((ii[:, None] < ii[None, :]) & blk).astype(np.float32)
    flow = -((ii[:, None] > ii[None, :]) & ~blk).astype(np.float32)
    triuI = (ii[:, None] <= ii[None, :]).astype(np.float32)
    ones = np.ones((128, 128), np.float32)
    return np.concatenate([ident, -ident, bdl, bdu, flow, triuI, ones],
                          axis=1).astype(np.float16)


def _get_compiled():
    key = ("nc", SILU_NATIVE, DEBUG_DUMP, PHASE_LIMIT, STAGE_LIMIT)
    if key not in _CACHE:
        _CACHE[key] = _build_bass()
    return _CACHE[key]


def _make_in_maps(hidden_states, Wq, Wk, Wv, conv_wq, conv_wk, conv_wv,
                  onorm_w, Wo):
    hidden_states = np.asarray(hidden_states, np.float32)
    Wq = np.asarray(Wq, np.float32)
    Wk = np.asarray(Wk, np.float32)
    Wv = np.asarray(Wv, np.float32)
    Wo = np.asarray(Wo, np.float32)
    conv_wq = np.asarray(conv_wq, np.float32)
    conv_wk = np.asarray(conv_wk, np.float32)
    conv_wv = np.asarray(conv_wv, np.float32)
    onorm_w = np.asarray(onorm_w, np.float32)

    consts = _make_consts()
    Wo_eff = (Wo * np.tile(onorm_w, H)[:, None]).astype(np.float16)

    def pack_w(Wcols):  # [D, CG] -> [KT8, 128, CG] fp16
        return np.ascontiguousarray(
            Wcols.astype(np.float16).reshape(KT8, 128, CG))

    def pack_x(xT):     # [D, T] -> [KT8, 128, T] fp16
        return np.ascontiguousarray(
            xT.astype(np.float16).reshape(KT8, 128, T))

    def pack_wo(Wrows):  # [CG, D] -> [4, 128, D] fp16
        return np.ascontiguousarray(
            Wrows.astype(np.float16).reshape(4, 128, D))

    def make_diag(cw):   # [CG, K] -> [CT, 128, K*128] fp16 diag
        o = np.zeros((CT, 128, CONV_K * 128), np.float16)
        for ct in range(CT):
            for tap in range(CONV_K):
                np.fill_diagonal(o[ct, :, tap * 128:(tap + 1) * 128],
                                 cw[ct * 128:(ct + 1) * 128, tap
                                    ].astype(np.float16))
        return o

    in_maps = []
    for core in range(NCORES):
        b, g = divmod(core, 2)
        cols = slice(CG * g, CG * (g + 1))
        in_maps.append({
            "x8": pack_x(np.ascontiguousarray(hidden_states[b].T)),
            "w8": np.concatenate([pack_w(Wq[:, cols]), pack_w(Wk[:, cols]),
                                  pack_w(Wv[:, cols])], axis=0),
            "wo8": pack_wo(Wo_eff[cols, :]),
            "diag": np.concatenate([make_diag(conv_wq[cols]),
                                    make_diag(conv_wk[cols])], axis=0),
            "cwv": np.ascontiguousarray(
                conv_wv[cols].reshape(CT, 128, CONV_K)).astype(np.float32),
            "consts": consts,
        })
    return in_maps


def kernel(hidden_states, Wq, Wk, Wv, conv_wq, conv_wk, conv_wv, onorm_w, Wo):
    from concourse.bass_utils import run_bass_kernel_spmd

    in_maps = _make_in_maps(hidden_states, Wq, Wk, Wv, conv_wq, conv_wk,
                            conv_wv, onorm_w, Wo)
    nc = _get_compiled()
    res = run_bass_kernel_spmd(nc, in_maps, core_ids=list(range(NCORES)),
                               **_CACHE.get("run_kwargs", {}))
    _CACHE["last_results"] = res
    out = np.zeros((B, T, D), np.float32)
    for core in range(NCORES):
        out[core // 2] += res.results[core]["out"].astype(np.float32)
    return out
